# revision 1
# baseline (speedup 1.0000x reference)
"""2-layer GCN encoder on two graphs, distributed over 8 Trainium2 NeuronCores.

Strategy
--------
Graph a -> cores 0-3, graph b -> cores 4-7. Each core owns 12,500 destination
nodes (relabeled for load balance) grouped into 196 ranges of 64 node slots.
Per range, in-edges are split by source-half (src < 25000 vs >=, because the
dma_gather index dtype is int16) and padded to 5 blocks of 128 edges per half.

Per 128-edge block the core gathers the 128 source rows (512 B each) with
dma_gather (SWDGE, 4 queues round-robin), builds a [128 edges x 64 dst] norm-
scaled selection matrix on the vector engine (fused is_equal*norm
tensor_scalar against a constant iota row), and accumulates
M^T @ S -> psum[feat, dst] on the tensor engine (fp32). Every destination
slot's result lands in a psum range tile; 4 ranges form a 256-column group
that flows through the dense chain W1 -> (+b) relu -> W2 on chip.

The same compiled program serves both GCN layers:
  launch A: table = x, weights (W1, b1, W2)      -> emits g = relu(xW1 agg) W2
  launch B: table = g, weights ([I|0], b2, [I;0]) -> emits relu(agg(g) + b2)
Aggregation uses the identity  A_hat (x W) == (A_hat x) W  so the sparse part
always runs at 128 features. Host does index prep / unpermute only.
"""

import os
import numpy as np

os.environ.setdefault("JAX_COMPILATION_CACHE_DIR", "/tmp/jax_cache")

import jax  # noqa: E402

try:
    jax.config.update("jax_compilation_cache_dir", "/tmp/jax_cache")
    jax.config.update("jax_persistent_cache_min_compile_time_secs", 0.0)
except Exception:
    pass

import concourse.bacc as bacc  # noqa: E402
import concourse.tile as tile  # noqa: E402
import concourse.mybir as mybir  # noqa: E402
from concourse.bass_utils import run_bass_kernel_spmd  # noqa: E402

# ---- static problem geometry (hardcoded per contract) ----
N_NODES = 50000
E_EDGES = 800000
D_IN = 128
D_HID = 256
HALF = 25000

N_CORES = 8
CORES_PER_GRAPH = 4
NODES_PER_CORE = N_NODES // CORES_PER_GRAPH  # 12500

DTILE = 64                 # dst slots per range
R = 196                    # ranges per core (196*64 = 12544 slots >= 12500)
NBLK_H = 5                 # 128-edge blocks per (range, half)
CAP_H = NBLK_H * 128       # 640 edge slots per (range, half)
NCALLS = R * 2             # one dma_gather call per (range, half)
NBLOCKS = NCALLS * NBLK_H  # 1960 S-blocks per core
IDXCOLS = CAP_H // 16      # 40 int16 columns per call
GROUPS = R // 4            # 49 dense groups of 4 ranges (256 dst slots)
MBUFS = 6                  # gather tile pool depth

META_W = DTILE + 2 * NBLOCKS  # iota | dstrel | norm

_prog = None


def _build_program():
    nc = bacc.Bacc("TRN2", target_bir_lowering=False, num_swdge_queues=4)
    f32 = mybir.dt.float32
    tbl = nc.declare_dram_parameter("tbl", [N_NODES, D_IN], f32, isOutput=False)
    idx = nc.declare_dram_parameter("idx", [128, NCALLS * IDXCOLS], mybir.dt.int16, isOutput=False)
    meta = nc.declare_dram_parameter("meta", [128, META_W], f32, isOutput=False)
    w1 = nc.declare_dram_parameter("w1", [128, D_HID], f32, isOutput=False)
    w2 = nc.declare_dram_parameter("w2", [D_HID, 128], f32, isOutput=False)
    b1c = nc.declare_dram_parameter("b1c", [128, 2], f32, isOutput=False)
    gout = nc.declare_dram_parameter("gout", [GROUPS, 128, 256], f32, isOutput=True)

    with tile.TileContext(nc) as tc:
        with (
            tc.tile_pool(name="res", bufs=1) as res,
            tc.tile_pool(name="mpool", bufs=MBUFS) as mp,
            tc.tile_pool(name="spool", bufs=4) as sp,
            tc.tile_pool(name="ssb", bufs=2) as ssb,
            tc.tile_pool(name="hsb", bufs=2) as hsb,
            tc.tile_pool(name="gsb", bufs=2) as gsbp,
            tc.tile_pool(name="psps", bufs=3, space="PSUM") as psps,
            tc.tile_pool(name="psh", bufs=2, space="PSUM") as psh,
            tc.tile_pool(name="psg", bufs=2, space="PSUM") as psg,
        ):
            idx_t = res.tile([128, NCALLS * IDXCOLS], mybir.dt.int16)
            nc.sync.dma_start(idx_t[:], idx[:, :])
            meta_t = res.tile([128, META_W], f32)
            nc.sync.dma_start(meta_t[:], meta[:, :])
            w1t = res.tile([128, D_HID], f32)
            nc.sync.dma_start(w1t[:], w1[:, :])
            w2t = res.tile([128, D_HID], f32)
            nc.sync.dma_start(w2t[:, 0:128], w2[0:128, :])
            nc.sync.dma_start(w2t[:, 128:256], w2[128:256, :])
            b1t = res.tile([128, 2], f32)
            nc.sync.dma_start(b1t[:], b1c[:, :])

            iota_ap = meta_t[:, 0:DTILE]

            for q in range(GROUPS):
                s_sb = ssb.tile([128, 256], f32, tag="s_sb")
                for rr in range(4):
                    r = q * 4 + rr
                    ps = psps.tile([128, DTILE], f32, tag="ps")
                    for h in range(2):
                        call = r * 2 + h
                        m = mp.tile([128, CAP_H], f32, tag="m")
                        nc.gpsimd.dma_gather(
                            out_ap=m[:].rearrange("p (b e) -> p b e", e=D_IN),
                            in_ap=tbl[h * HALF:(h + 1) * HALF, :],
                            idxs_ap=idx_t[:, call * IDXCOLS:(call + 1) * IDXCOLS],
                            num_idxs=CAP_H,
                            num_idxs_reg=CAP_H,
                            elem_size=D_IN,
                            single_packet=False,
                            queue_num=call % 4,
                        )
                        for b in range(NBLK_H):
                            col = call * NBLK_H + b
                            s = sp.tile([128, DTILE], f32, tag="s")
                            nc.vector.tensor_scalar(
                                out=s[:],
                                in0=iota_ap,
                                scalar1=meta_t[:, DTILE + col:DTILE + col + 1],
                                scalar2=meta_t[:, DTILE + NBLOCKS + col:DTILE + NBLOCKS + col + 1],
                                op0=mybir.AluOpType.is_equal,
                                op1=mybir.AluOpType.mult,
                            )
                            nc.tensor.matmul(
                                out=ps[:],
                                lhsT=m[:, b * 128:(b + 1) * 128],
                                rhs=s[:],
                                start=(h == 0 and b == 0),
                                stop=(h == 1 and b == NBLK_H - 1),
                            )
                    nc.vector.tensor_copy(s_sb[:, rr * DTILE:(rr + 1) * DTILE], ps[:])

                h1ps = psh.tile([128, 512], f32, tag="h1ps")
                nc.tensor.matmul(out=h1ps[:, 0:256], lhsT=w1t[:, 0:128], rhs=s_sb[:], start=True, stop=True)
                nc.tensor.matmul(out=h1ps[:, 256:512], lhsT=w1t[:, 128:256], rhs=s_sb[:], start=True, stop=True)
                h1 = hsb.tile([128, 512], f32, tag="h1")
                nc.scalar.activation(h1[:, 0:256], h1ps[:, 0:256], mybir.ActivationFunctionType.Relu, bias=b1t[:, 0:1])
                nc.scalar.activation(h1[:, 256:512], h1ps[:, 256:512], mybir.ActivationFunctionType.Relu, bias=b1t[:, 1:2])
                gps = psg.tile([128, 256], f32, tag="gps")
                nc.tensor.matmul(out=gps[:], lhsT=w2t[:, 0:128], rhs=h1[:, 0:256], start=True, stop=False)
                nc.tensor.matmul(out=gps[:], lhsT=w2t[:, 128:256], rhs=h1[:, 256:512], start=False, stop=True)
                gsb = gsbp.tile([128, 256], f32, tag="gsb")
                nc.vector.tensor_copy(gsb[:], gps[:])
                nc.sync.dma_start(gout[q], gsb[:])

    nc.compile()
    return nc


def _get_program():
    global _prog
    if _prog is None:
        _prog = _build_program()
    return _prog


def _preprocess_graph(edge):
    """Per graph: per-core packing. Returns list of 4 core dicts + dinv."""
    src = np.asarray(edge[0], np.int64)
    dst = np.asarray(edge[1], np.int64)
    deg = np.bincount(dst, minlength=N_NODES).astype(np.float32)
    dinv = (1.0 / np.sqrt(deg + np.float32(1.0))).astype(np.float32)

    # append self loops
    selfs = np.arange(N_NODES, dtype=np.int64)
    asrc = np.concatenate([src, selfs])
    adst = np.concatenate([dst, selfs])
    anorm = (dinv[asrc] * dinv[adst]).astype(np.float32)

    cores = []
    for c in range(CORES_PER_GRAPH):
        lo, hi = c * NODES_PER_CORE, (c + 1) * NODES_PER_CORE
        emask = (adst >= lo) & (adst < hi)
        es = asrc[emask]
        ed = adst[emask] - lo
        en = anorm[emask]
        eh = (es >= HALF).astype(np.int64)

        # per-node degree by half
        degh = np.zeros((NODES_PER_CORE, 2), np.int64)
        np.add.at(degh, (ed, eh), 1)

        # --- pack nodes into R bins of <=64, per-half load <= CAP_H ---
        order = np.argsort(-(degh[:, 0] + degh[:, 1]), kind="stable")
        bin_of = np.empty(NODES_PER_CORE, np.int32)
        # snake deal
        k = 0
        direction = 1
        pos = 0
        for v in order:
            bin_of[v] = pos
            k += 1
            if direction == 1:
                if pos == R - 1:
                    direction = -1
                else:
                    pos += 1
            else:
                if pos == 0:
                    direction = 1
                else:
                    pos -= 1
        # loads + capacity repair
        binload = np.zeros((R, 2), np.int64)
        np.add.at(binload, (bin_of, np.zeros_like(bin_of)), 0)
        for hh in range(2):
            np.add.at(binload[:, hh], bin_of, degh[:, hh])
        bincnt = np.bincount(bin_of, minlength=R)
        for _ in range(2000):
            over = np.where((binload[:, 0] > CAP_H) | (binload[:, 1] > CAP_H))[0]
            if len(over) == 0:
                break
            bo = over[0]
            hh = 0 if binload[bo, 0] > CAP_H else 1
            # heaviest (by hh) node in bo, swap with lightest node of the
            # least-loaded bin
            cand = np.where(bin_of == bo)[0]
            vheavy = cand[np.argmax(degh[cand, hh])]
            bl = int(np.argmin(binload[:, hh]))
            cand2 = np.where(bin_of == bl)[0]
            vlight = cand2[np.argmin(degh[cand2, hh])]
            bin_of[vheavy], bin_of[vlight] = bl, bo
            for h2 in range(2):
                binload[bo, h2] += degh[vlight, h2] - degh[vheavy, h2]
                binload[bl, h2] += degh[vheavy, h2] - degh[vlight, h2]
        assert (binload <= CAP_H).all(), f"bin packing failed: {binload.max(0)}"
        assert (bincnt <= DTILE).all()

        # position of each node within its bin
        order2 = np.lexsort((np.arange(NODES_PER_CORE), bin_of))
        pos_in_bin = np.empty(NODES_PER_CORE, np.int64)
        binstart = np.zeros(R + 1, np.int64)
        np.cumsum(np.bincount(bin_of, minlength=R), out=binstart[1:])
        pos_in_bin[order2] = np.arange(NODES_PER_CORE) - binstart[bin_of[order2]]

        # column map: group q, col cidx -> global node id (or -1)
        cols_map = np.full((GROUPS, 256), -1, np.int64)
        gq = bin_of // 4
        gcol = (bin_of % 4) * DTILE + pos_in_bin
        cols_map[gq, gcol] = np.arange(lo, hi)

        # --- edge slot assembly ---
        gidx = bin_of[ed] * 2 + eh            # call index per edge
        okey = np.lexsort((np.arange(len(es)), gidx))
        gsorted = gidx[okey]
        counts = np.bincount(gsorted, minlength=NCALLS)
        assert counts.max() <= CAP_H
        starts = np.zeros(NCALLS + 1, np.int64)
        np.cumsum(counts, out=starts[1:])
        within = np.arange(len(es)) - starts[gsorted]
        slot = gsorted * CAP_H + within

        # pads gather row 0 with dstrel=-1 (S column 0 -> contribution 0);
        # every index stays valid so num_idxs_reg == valid count.
        total = NCALLS * CAP_H
        idx_slots = np.zeros(total, np.int64)
        dst_slots = np.full(total, -1.0, np.float32)
        nrm_slots = np.zeros(total, np.float32)
        idx_slots[slot] = es[okey] - eh[okey] * HALF
        dst_slots[slot] = pos_in_bin[ed[okey]].astype(np.float32)
        nrm_slots[slot] = en[okey]

        a = idx_slots.reshape(NCALLS, IDXCOLS, 16)
        idx16 = np.tile(
            np.ascontiguousarray(np.transpose(a, (2, 0, 1))).reshape(16, NCALLS * IDXCOLS),
            (8, 1),
        ).astype(np.int16)
        bblocks = dst_slots.reshape(NBLOCKS, 128)
        dstrel = np.ascontiguousarray(bblocks.T)  # [128, NBLOCKS]
        nb = nrm_slots.reshape(NBLOCKS, 128)
        norm = np.ascontiguousarray(nb.T)
        iota = np.broadcast_to(np.arange(DTILE, dtype=np.float32), (128, DTILE))
        meta = np.concatenate([iota, dstrel, norm], axis=1).astype(np.float32)

        cores.append({"idx": idx16, "meta": meta, "cols_map": cols_map})
    return cores


def _assemble(results, cores_a, cores_b):
    """Gather per-core gout into full [N, 128] arrays for each graph."""
    outs = []
    for g, cores in ((0, cores_a), (1, cores_b)):
        full = np.zeros((N_NODES, D_IN), np.float32)
        for c in range(CORES_PER_GRAPH):
            go = results[g * CORES_PER_GRAPH + c]["gout"]  # [GROUPS, 128, 256]
            cm = cores[c]["cols_map"]
            for q in range(GROUPS):
                valid = cm[q] >= 0
                full[cm[q][valid]] = go[q][:, valid].T
        outs.append(full)
    return outs


def _spot_check(full, tbl, edge, dinv, post, n_samples=24, tol=2e-3):
    """Verify a few random nodes of a launch output on host (numpy)."""
    src = np.asarray(edge[0], np.int64)
    dst = np.asarray(edge[1], np.int64)
    rng = np.random.default_rng(12345)
    nodes = rng.integers(0, N_NODES, size=n_samples)
    for v in nodes:
        ine = np.where(dst == v)[0]
        s = (dinv[src[ine]] * dinv[v])[:, None] * tbl[src[ine]]
        s = s.sum(axis=0, dtype=np.float64) + np.float64(dinv[v]) ** 2 * tbl[v]
        exp = post(s)
        got = full[v]
        scale = max(np.abs(exp).max(), 1e-3)
        if np.abs(got - exp).max() / scale > tol:
            return False
    return True


def kernel(x_a, edge_a, x_b, edge_b, W1, b1, W2, b2):
    x_a = np.ascontiguousarray(np.asarray(x_a, np.float32))
    x_b = np.ascontiguousarray(np.asarray(x_b, np.float32))
    W1 = np.asarray(W1, np.float32)
    b1 = np.asarray(b1, np.float32)
    W2 = np.asarray(W2, np.float32)
    b2 = np.asarray(b2, np.float32)

    nc = _get_program()
    cores_a = _preprocess_graph(np.asarray(edge_a))
    cores_b = _preprocess_graph(np.asarray(edge_b))

    b1c = np.stack([b1[0:128], b1[128:256]], axis=1).astype(np.float32)
    eye = np.eye(128, dtype=np.float32)
    w1_id = np.concatenate([eye, np.zeros((128, 128), np.float32)], axis=1)
    w2_id = np.concatenate([eye, np.zeros((128, 128), np.float32)], axis=0)
    b1c_id = np.stack([b2, np.zeros(128, np.float32)], axis=1).astype(np.float32)

    def maps(tbl_a, tbl_b, w1m, w2m, b1m):
        ms = []
        for g, (tbl, cores) in enumerate(((tbl_a, cores_a), (tbl_b, cores_b))):
            for c in range(CORES_PER_GRAPH):
                ms.append({
                    "tbl": tbl,
                    "idx": cores[c]["idx"],
                    "meta": cores[c]["meta"],
                    "w1": w1m, "w2": w2m, "b1c": b1m,
                })
        return ms

    core_ids = list(range(N_CORES))

    def run(in_maps):
        import time as _t
        last = None
        for attempt in range(4):
            try:
                t0 = _t.time()
                res = run_bass_kernel_spmd(nc, in_maps, core_ids)
                LAUNCH_WALL.append(_t.time() - t0)
                return res
            except Exception as e:  # wedged core recovers on retry
                last = e
                _t.sleep(5)
        raise last

    dinv_a = (1.0 / np.sqrt(np.bincount(np.asarray(edge_a[1], np.int64),
              minlength=N_NODES).astype(np.float32) + 1.0)).astype(np.float32)
    dinv_b = (1.0 / np.sqrt(np.bincount(np.asarray(edge_b[1], np.int64),
              minlength=N_NODES).astype(np.float32) + 1.0)).astype(np.float32)

    def post_a(s):
        return np.maximum(s @ W1.astype(np.float64) + b1, 0.0) @ W2.astype(np.float64)

    def post_b(s):
        return np.maximum(s + b2, 0.0)

    # run each launch until the host spot-check passes (guards against rare
    # silent device-side corruption)
    for attempt in range(4):
        resA = run(maps(x_a, x_b, W1, W2, b1c))
        g_a, g_b = _assemble(resA.results, cores_a, cores_b)
        if (_spot_check(g_a, x_a, edge_a, dinv_a, post_a)
                and _spot_check(g_b, x_b, edge_b, dinv_b, post_a)):
            break
    for attempt in range(4):
        resB = run(maps(g_a, g_b, w1_id, w2_id, b1c_id))
        z_a, z_b = _assemble(resB.results, cores_a, cores_b)
        if (_spot_check(z_a, g_a, edge_a, dinv_a, post_b)
                and _spot_check(z_b, g_b, edge_b, dinv_b, post_b)):
            break
    return (z_a, z_b)


LAUNCH_WALL = []



# revision 2
# speedup vs baseline: 1.9373x; 1.9373x over previous
"""2-layer GCN encoder on two graphs, distributed over 8 Trainium2 NeuronCores.

Strategy (v2)
-------------
Graph a -> cores 0-3, graph b -> cores 4-7. Each core owns 12,500 destination
nodes (original-id contiguous chunk) packed into 100 bins of <=128 dst slots,
balanced so every (bin, src-half) in-edge load fits 1024 slots (8 blocks of
128). Self-loops are NOT materialized as edges: the self term is added from a
host-prescaled transposed own-chunk block (dinv^2 * x_own)^T resident in SBUF.

Gathers are batched: ONE dma_gather per (group of 2 bins, half) = 2048 rows of
256 B (bf16), so SWDGE descriptor-gen fixed cost (994 ns/call) amortizes over
2048 descriptors. Tables are bf16, aggregation matmuls run bf16 (1 cyc/col vs
4 for fp32), psum accumulates fp32.

Per 128-edge block the core builds a [128 e x 128 dst] norm-scaled selection
matrix on the vector engine (is_equal*norm against an iota row) and
accumulates M^T @ S -> psum[feat, dst]. Two bins form a 256-column group that
flows through the dense chain W1 -> (+b1) relu -> W2 on chip (all bf16 in,
fp32 psum). The same compiled program serves both GCN layers:
  launch A: table = x  (bf16), weights (W1, b1, W2)       -> g
  launch B: table = g  (bf16), weights ([I|0], b2, [I;0]) -> z = relu(agg+b2)
using A_hat (x W) == (A_hat x) W so the sparse part always runs at 128
features. Host does packing / index prep / unpermute only.
"""

import os
import numpy as np

os.environ.setdefault("JAX_COMPILATION_CACHE_DIR", "/tmp/jax_cache")

import jax  # noqa: E402

try:
    jax.config.update("jax_compilation_cache_dir", "/tmp/jax_cache")
    jax.config.update("jax_persistent_cache_min_compile_time_secs", 0.0)
except Exception:
    pass

import ml_dtypes  # noqa: E402
import concourse.bacc as bacc  # noqa: E402
import concourse.tile as tile  # noqa: E402
import concourse.mybir as mybir  # noqa: E402
from concourse.bass_utils import run_bass_kernel_spmd  # noqa: E402

# ---- static problem geometry (hardcoded per contract) ----
N_NODES = 50000
D_IN = 128
D_HID = 256
HALF = 25000

N_CORES = 8
CORES_PER_GRAPH = 4
NPC = N_NODES // CORES_PER_GRAPH  # 12500 nodes per core

R = 100                    # bins per core
DTILE = 128                # dst slots per bin
NBLK_H = 8                 # 128-edge blocks per (bin, half)
CAP_H = NBLK_H * 128       # 1024 edge slots per (bin, half)
GROUPS = R // 2            # 50 dense groups of 2 bins (256 dst slots)
NCALLS = GROUPS * 2        # one dma_gather per (group, half): 2048 idx
CALL_IDX = 2 * CAP_H       # 2048 rows per gather call
IDXCOLS = CALL_IDX // 16   # 128 int16 cols per call
NBLOCKS = R * 2 * NBLK_H   # 1600 S-blocks per core
NSLOTS = R * DTILE         # 12800 dst slots per core
MBUFS = 6

BF16 = ml_dtypes.bfloat16

_prog = None


def _build_program():
    nc = bacc.Bacc("TRN2", target_bir_lowering=False, num_swdge_queues=4)
    f32 = mybir.dt.float32
    bf16 = mybir.dt.bfloat16
    tbl = nc.declare_dram_parameter("tbl", [N_NODES, D_IN], bf16, isOutput=False)
    idx = nc.declare_dram_parameter("idx", [128, NCALLS * IDXCOLS], mybir.dt.int16, isOutput=False)
    meta = nc.declare_dram_parameter("meta", [128, 2 * NBLOCKS], f32, isOutput=False)  # interleaved (dstrel, norm) per block
    iota = nc.declare_dram_parameter("iota", [128, DTILE], bf16, isOutput=False)
    selfp = nc.declare_dram_parameter("selfp", [128, NSLOTS], bf16, isOutput=False)
    w1 = nc.declare_dram_parameter("w1", [128, D_HID], bf16, isOutput=False)
    w2 = nc.declare_dram_parameter("w2", [D_HID, 128], bf16, isOutput=False)
    b1c = nc.declare_dram_parameter("b1c", [128, 2], f32, isOutput=False)
    gout = nc.declare_dram_parameter("gout", [GROUPS, 128, 256], bf16, isOutput=True)

    with tile.TileContext(nc) as tc:
        with (
            tc.tile_pool(name="res", bufs=1) as res,
            tc.tile_pool(name="mpool", bufs=MBUFS) as mp,
            tc.tile_pool(name="spool", bufs=4) as sp,
            tc.tile_pool(name="ssb", bufs=2) as ssb,
            tc.tile_pool(name="hsb", bufs=2) as hsb,
            tc.tile_pool(name="gsb", bufs=2) as gsbp,
            tc.tile_pool(name="psps", bufs=3, space="PSUM") as psps,
            tc.tile_pool(name="psh", bufs=2, space="PSUM") as psh,
            tc.tile_pool(name="psg", bufs=2, space="PSUM") as psg,
        ):
            idx_t = res.tile([128, NCALLS * IDXCOLS], mybir.dt.int16)
            nc.sync.dma_start(idx_t[:], idx[:, :])
            meta_t = res.tile([128, 2 * NBLOCKS], f32)
            nc.sync.dma_start(meta_t[:], meta[:, :])
            iota_t = res.tile([128, DTILE], bf16)
            nc.sync.dma_start(iota_t[:], iota[:, :])
            self_t = res.tile([128, NSLOTS], bf16)
            nc.sync.dma_start(self_t[:], selfp[:, :])
            w1t = res.tile([128, D_HID], bf16)
            nc.sync.dma_start(w1t[:], w1[:, :])
            w2t = res.tile([128, D_HID], bf16)
            nc.sync.dma_start(w2t[:, 0:128], w2[0:128, :])
            nc.sync.dma_start(w2t[:, 128:256], w2[128:256, :])
            b1t = res.tile([128, 2], f32)
            nc.sync.dma_start(b1t[:], b1c[:, :])

            iota_ap = iota_t[:]

            for q in range(GROUPS):
                ms = []
                for h in range(2):
                    call = q * 2 + h
                    m = mp.tile([128, CALL_IDX], bf16, tag="m")
                    nc.gpsimd.dma_gather(
                        out_ap=m[:].rearrange("p (b e) -> p b e", e=D_IN),
                        in_ap=tbl[h * HALF:(h + 1) * HALF, :],
                        idxs_ap=idx_t[:, call * IDXCOLS:(call + 1) * IDXCOLS],
                        num_idxs=CALL_IDX,
                        num_idxs_reg=CALL_IDX,
                        elem_size=D_IN,
                        single_packet=False,
                        queue_num=call % 4,
                    )
                    ms.append(m)
                s_sb = ssb.tile([128, 256], bf16, tag="s_sb")
                for bb in range(2):
                    r = q * 2 + bb
                    ps = psps.tile([128, DTILE], f32, tag="ps")
                    for h in range(2):
                        for b in range(NBLK_H):
                            blkcol = ((q * 2 + h) * 2 + bb) * NBLK_H + b
                            s = sp.tile([128, DTILE], bf16, tag="s")
                            nc.vector.tensor_scalar(
                                out=s[:],
                                in0=iota_ap,
                                scalar1=meta_t[:, 2 * blkcol:2 * blkcol + 1],
                                scalar2=meta_t[:, 2 * blkcol + 1:2 * blkcol + 2],
                                op0=mybir.AluOpType.is_equal,
                                op1=mybir.AluOpType.mult,
                            )
                            nc.tensor.matmul(
                                out=ps[:],
                                lhsT=ms[h][:, (bb * NBLK_H + b) * 128:(bb * NBLK_H + b + 1) * 128],
                                rhs=s[:],
                                start=(h == 0 and b == 0),
                                stop=(h == 1 and b == NBLK_H - 1),
                            )
                    nc.vector.tensor_tensor(
                        out=s_sb[:, bb * DTILE:(bb + 1) * DTILE],
                        in0=ps[:],
                        in1=self_t[:, r * DTILE:(r + 1) * DTILE],
                        op=mybir.AluOpType.add,
                    )

                h1ps = psh.tile([128, 512], f32, tag="h1ps")
                nc.tensor.matmul(out=h1ps[:, 0:256], lhsT=w1t[:, 0:128], rhs=s_sb[:], start=True, stop=True)
                nc.tensor.matmul(out=h1ps[:, 256:512], lhsT=w1t[:, 128:256], rhs=s_sb[:], start=True, stop=True)
                h1 = hsb.tile([128, 512], bf16, tag="h1")
                nc.scalar.activation(h1[:, 0:256], h1ps[:, 0:256], mybir.ActivationFunctionType.Relu, bias=b1t[:, 0:1])
                nc.scalar.activation(h1[:, 256:512], h1ps[:, 256:512], mybir.ActivationFunctionType.Relu, bias=b1t[:, 1:2])
                gps = psg.tile([128, 256], f32, tag="gps")
                nc.tensor.matmul(out=gps[:], lhsT=w2t[:, 0:128], rhs=h1[:, 0:256], start=True, stop=False)
                nc.tensor.matmul(out=gps[:], lhsT=w2t[:, 128:256], rhs=h1[:, 256:512], start=False, stop=True)
                gsb = gsbp.tile([128, 256], bf16, tag="gsb")
                nc.vector.tensor_copy(gsb[:], gps[:])
                nc.sync.dma_start(gout[q], gsb[:])

    nc.compile()
    return nc


def _get_program():
    global _prog
    if _prog is None:
        _prog = _build_program()
    return _prog


def _pack_core(deg2):
    """Greedy balance 12,500 nodes into R bins: <=DTILE nodes/bin and both
    per-half edge loads <= CAP_H. deg2: [NPC, 2]."""
    tot = deg2[:, 0] + deg2[:, 1]
    order = np.argsort(-tot, kind="stable")
    binload = np.zeros((R, 2), np.int64)
    bincnt = np.zeros(R, np.int64)
    bin_of = np.full(NPC, -1, np.int32)
    for v in order:
        d0, d1 = deg2[v]
        ok = (bincnt < DTILE) & (binload[:, 0] + d0 <= CAP_H) & (binload[:, 1] + d1 <= CAP_H)
        if not ok.any():
            return None
        score = np.maximum(binload[:, 0] + d0, binload[:, 1] + d1)
        score = np.where(ok, score, 1 << 30)
        b = int(np.argmin(score))
        bin_of[v] = b
        bincnt[b] += 1
        binload[b, 0] += d0
        binload[b, 1] += d1
    return bin_of


def _preprocess_graph(edge):
    """Per graph: per-core packing + slot assembly. Returns (cores, dinv)."""
    src = np.asarray(edge[0], np.int64)
    dst = np.asarray(edge[1], np.int64)
    deg = np.bincount(dst, minlength=N_NODES).astype(np.float32)
    dinv = (1.0 / np.sqrt(deg + np.float32(1.0))).astype(np.float32)
    anorm = (dinv[src] * dinv[dst]).astype(np.float32)
    ehalf = (src >= HALF).astype(np.int64)

    cores = []
    for c in range(CORES_PER_GRAPH):
        lo, hi = c * NPC, (c + 1) * NPC
        emask = (dst >= lo) & (dst < hi)
        es = src[emask]
        ed = dst[emask] - lo
        en = anorm[emask]
        eh = ehalf[emask]

        deg2 = np.zeros((NPC, 2), np.int64)
        np.add.at(deg2, (ed, eh), 1)
        bin_of = _pack_core(deg2)
        assert bin_of is not None, "bin packing failed"

        # position of each node within its bin
        order2 = np.lexsort((np.arange(NPC), bin_of))
        pos_in_bin = np.empty(NPC, np.int64)
        binstart = np.zeros(R + 1, np.int64)
        np.cumsum(np.bincount(bin_of, minlength=R), out=binstart[1:])
        pos_in_bin[order2] = np.arange(NPC) - binstart[bin_of[order2]]

        # column map: group q, col -> global node id (or -1); self slot rows
        cols_map = np.full((GROUPS, 256), -1, np.int64)
        q_of = bin_of // 2
        col_of = (bin_of % 2) * DTILE + pos_in_bin
        cols_map[q_of, col_of] = np.arange(lo, hi)
        self_rows = np.full(NSLOTS, -1, np.int64)
        self_rows[bin_of * DTILE + pos_in_bin] = np.arange(lo, hi)

        # --- edge slot assembly: stream k = ((q*2 + h)*2 + bb), cap 1024 ---
        k = (q_of[ed] * 2 + eh) * 2 + (bin_of[ed] % 2)
        okey = np.lexsort((np.arange(len(es)), k))
        ksorted = k[okey]
        counts = np.bincount(ksorted, minlength=R * 2)
        assert counts.max() <= CAP_H
        starts = np.zeros(R * 2 + 1, np.int64)
        np.cumsum(counts, out=starts[1:])
        within = np.arange(len(es)) - starts[ksorted]
        slot = ksorted * CAP_H + within

        # pad slots gather row 0 with dstrel=-1 (S column contribution 0)
        total = NCALLS * CALL_IDX
        idx_slots = np.zeros(total, np.int64)
        dst_slots = np.full(total, -1.0, np.float32)
        nrm_slots = np.zeros(total, np.float32)
        idx_slots[slot] = es[okey] - eh[okey] * HALF
        dst_slots[slot] = pos_in_bin[ed[okey]].astype(np.float32)
        nrm_slots[slot] = en[okey]

        a = idx_slots.reshape(NCALLS, IDXCOLS, 16)
        idx16 = np.tile(
            np.ascontiguousarray(np.transpose(a, (2, 0, 1))).reshape(16, NCALLS * IDXCOLS),
            (8, 1),
        ).astype(np.int16)
        pair = np.stack([dst_slots.reshape(NBLOCKS, 128),
                         nrm_slots.reshape(NBLOCKS, 128)], axis=1)  # [NBLOCKS, 2, 128]
        meta = np.ascontiguousarray(pair.reshape(NBLOCKS * 2, 128).T).astype(np.float32)

        cores.append({
            "idx": idx16, "meta": meta, "cols_map": cols_map,
            "self_rows": self_rows, "lo": lo,
        })
    return cores, dinv


def _self_block(core, tbl_f32, dinv):
    """[128, NSLOTS] bf16: column (bin*128+pos) = dinv^2[v] * tbl[v]."""
    sr = core["self_rows"]
    valid = sr >= 0
    blk = np.zeros((NSLOTS, D_IN), np.float32)
    v = sr[valid]
    blk[valid] = tbl_f32[v] * (dinv[v] * dinv[v])[:, None]
    return np.ascontiguousarray(blk.T).astype(BF16)


def _assemble(results, cores_list):
    """Gather per-core gout into full [N, 128] fp32 arrays for each graph."""
    outs = []
    for g, cores in enumerate(cores_list):
        full = np.zeros((N_NODES, D_IN), np.float32)
        for c in range(CORES_PER_GRAPH):
            go = np.asarray(results[g * CORES_PER_GRAPH + c]["gout"], dtype=np.float32)
            cm = cores[c]["cols_map"]
            for q in range(GROUPS):
                valid = cm[q] >= 0
                full[cm[q][valid]] = go[q][:, valid].T
        outs.append(full)
    return outs


def _spot_check(full, tbl, edge, dinv, post, n_samples=24, tol=5e-2):
    """Verify a few random nodes of a launch output on host (numpy)."""
    src = np.asarray(edge[0], np.int64)
    dst = np.asarray(edge[1], np.int64)
    rng = np.random.default_rng(12345)
    nodes = rng.integers(0, N_NODES, size=n_samples)
    for v in nodes:
        ine = np.where(dst == v)[0]
        s = (dinv[src[ine]] * dinv[v])[:, None] * tbl[src[ine]]
        s = s.sum(axis=0, dtype=np.float64) + np.float64(dinv[v]) ** 2 * tbl[v]
        exp = post(s)
        got = full[v]
        scale = max(np.abs(exp).max(), 1e-3)
        if np.abs(got - exp).max() / scale > tol:
            return False
    return True


LAUNCH_WALL = []
IOTA = np.ascontiguousarray(
    np.broadcast_to(np.arange(DTILE, dtype=np.float32), (128, DTILE))).astype(BF16)


def kernel(x_a, edge_a, x_b, edge_b, W1, b1, W2, b2):
    x_a = np.ascontiguousarray(np.asarray(x_a, np.float32))
    x_b = np.ascontiguousarray(np.asarray(x_b, np.float32))
    W1 = np.asarray(W1, np.float32)
    b1 = np.asarray(b1, np.float32)
    W2 = np.asarray(W2, np.float32)
    b2 = np.asarray(b2, np.float32)

    nc = _get_program()
    cores_a, dinv_a = _preprocess_graph(np.asarray(edge_a))
    cores_b, dinv_b = _preprocess_graph(np.asarray(edge_b))

    b1c = np.stack([b1[0:128], b1[128:256]], axis=1).astype(np.float32)
    eye = np.eye(128, dtype=np.float32)
    w1_id = np.concatenate([eye, np.zeros((128, 128), np.float32)], axis=1).astype(BF16)
    w2_id = np.concatenate([eye, np.zeros((128, 128), np.float32)], axis=0).astype(BF16)
    b1c_id = np.stack([b2, np.zeros(128, np.float32)], axis=1).astype(np.float32)
    w1_b = W1.astype(BF16)
    w2_b = W2.astype(BF16)

    def maps(tbl_a, tbl_b, w1m, w2m, b1m):
        tba = tbl_a.astype(BF16)
        tbb = tbl_b.astype(BF16)
        ms = []
        for tb, tf, cores, dinv in ((tba, tbl_a, cores_a, dinv_a),
                                    (tbb, tbl_b, cores_b, dinv_b)):
            for c in range(CORES_PER_GRAPH):
                ms.append({
                    "tbl": tb,
                    "idx": cores[c]["idx"],
                    "meta": cores[c]["meta"],
                    "iota": IOTA,
                    "selfp": _self_block(cores[c], tf, dinv),
                    "w1": w1m, "w2": w2m, "b1c": b1m,
                })
        return ms

    core_ids = list(range(N_CORES))

    def run(in_maps):
        import time as _t
        last = None
        for attempt in range(4):
            try:
                t0 = _t.time()
                res = run_bass_kernel_spmd(nc, in_maps, core_ids)
                LAUNCH_WALL.append(_t.time() - t0)
                return res
            except Exception as e:  # wedged core recovers on retry
                last = e
                _t.sleep(5)
        raise last

    def post_a(s):
        return np.maximum(s @ W1.astype(np.float64) + b1, 0.0) @ W2.astype(np.float64)

    def post_b(s):
        return np.maximum(s + b2, 0.0)

    # run each launch until the host spot-check passes (guards against rare
    # silent device-side corruption)
    for attempt in range(4):
        resA = run(maps(x_a, x_b, w1_b, w2_b, b1c))
        g_a, g_b = _assemble(resA.results, (cores_a, cores_b))
        if (_spot_check(g_a, x_a, edge_a, dinv_a, post_a)
                and _spot_check(g_b, x_b, edge_b, dinv_b, post_a)):
            break
    for attempt in range(4):
        resB = run(maps(g_a, g_b, w1_id, w2_id, b1c_id))
        z_a, z_b = _assemble(resB.results, (cores_a, cores_b))
        if (_spot_check(z_a, g_a, edge_a, dinv_a, post_b)
                and _spot_check(z_b, g_b, edge_b, dinv_b, post_b)):
            break
    return (z_a, z_b)


# revision 3
# speedup vs baseline: 1.9632x; 1.0134x over previous
"""2-layer GCN encoder on two graphs, distributed over 8 Trainium2 NeuronCores.

Strategy (v2)
-------------
Graph a -> cores 0-3, graph b -> cores 4-7. Each core owns 12,500 destination
nodes (original-id contiguous chunk) packed into 100 bins of <=128 dst slots,
balanced so every (bin, src-half) in-edge load fits 1024 slots (8 blocks of
128). Self-loops are NOT materialized as edges: the self term is added from a
host-prescaled transposed own-chunk block (dinv^2 * x_own)^T resident in SBUF.

Gathers are batched: ONE dma_gather per (group of 2 bins, half) = 2048 rows of
256 B (bf16), so SWDGE descriptor-gen fixed cost (994 ns/call) amortizes over
2048 descriptors. Tables are bf16, aggregation matmuls run bf16 (1 cyc/col vs
4 for fp32), psum accumulates fp32.

Per 128-edge block the core builds a [128 e x 128 dst] norm-scaled selection
matrix on the vector engine (is_equal*norm against an iota row) and
accumulates M^T @ S -> psum[feat, dst]. Two bins form a 256-column group that
flows through the dense chain W1 -> (+b1) relu -> W2 on chip (all bf16 in,
fp32 psum). The same compiled program serves both GCN layers:
  launch A: table = x  (bf16), weights (W1, b1, W2)       -> g
  launch B: table = g  (bf16), weights ([I|0], b2, [I;0]) -> z = relu(agg+b2)
using A_hat (x W) == (A_hat x) W so the sparse part always runs at 128
features. Host does packing / index prep / unpermute only.
"""

import os
import numpy as np

os.environ.setdefault("JAX_COMPILATION_CACHE_DIR", "/tmp/jax_cache")

import jax  # noqa: E402

try:
    jax.config.update("jax_compilation_cache_dir", "/tmp/jax_cache")
    jax.config.update("jax_persistent_cache_min_compile_time_secs", 0.0)
except Exception:
    pass

import ml_dtypes  # noqa: E402
import concourse.bacc as bacc  # noqa: E402
import concourse.tile as tile  # noqa: E402
import concourse.mybir as mybir  # noqa: E402
from concourse.bass_utils import run_bass_kernel_spmd  # noqa: E402

# ---- static problem geometry (hardcoded per contract) ----
N_NODES = 50000
D_IN = 128
D_HID = 256
HALF = 25000

N_CORES = 8
CORES_PER_GRAPH = 4
NPC = N_NODES // CORES_PER_GRAPH  # 12500 nodes per core

R = 100                    # bins per core
DTILE = 128                # dst slots per bin
NBLK_H = 8                 # 128-edge blocks per (bin, half)
CAP_H = NBLK_H * 128       # 1024 edge slots per (bin, half)
GROUPS = R // 2            # 50 dense groups of 2 bins (256 dst slots)
NCALLS = GROUPS * 2        # one dma_gather per (group, half): 2048 idx
CALL_IDX = 2 * CAP_H       # 2048 rows per gather call
IDXCOLS = CALL_IDX // 16   # 128 int16 cols per call
NBLOCKS = R * 2 * NBLK_H   # 1600 S-blocks per core
NSLOTS = R * DTILE         # 12800 dst slots per core
MBUFS = 6

BF16 = ml_dtypes.bfloat16

_prog = None


def _build_program():
    nc = bacc.Bacc("TRN2", target_bir_lowering=False, num_swdge_queues=4)
    f32 = mybir.dt.float32
    bf16 = mybir.dt.bfloat16
    tbl = nc.declare_dram_parameter("tbl", [N_NODES, D_IN], bf16, isOutput=False)
    idx = nc.declare_dram_parameter("idx", [16, NCALLS * IDXCOLS], f32, isOutput=False)
    pat = nc.declare_dram_parameter("pat", [16, 128], f32, isOutput=False)
    meta = nc.declare_dram_parameter("meta", [128, 2 * NBLOCKS], f32, isOutput=False)  # interleaved (dstrel, norm) per block
    iota = nc.declare_dram_parameter("iota", [128, DTILE], bf16, isOutput=False)
    selfp = nc.declare_dram_parameter("selfp", [128, NSLOTS], bf16, isOutput=False)
    w1 = nc.declare_dram_parameter("w1", [128, D_HID], bf16, isOutput=False)
    w2 = nc.declare_dram_parameter("w2", [D_HID, 128], bf16, isOutput=False)
    b1c = nc.declare_dram_parameter("b1c", [128, 2], f32, isOutput=False)
    gout = nc.declare_dram_parameter("gout", [GROUPS, 128, 256], bf16, isOutput=True)

    with tile.TileContext(nc) as tc:
        with (
            tc.tile_pool(name="res", bufs=1) as res,
            tc.tile_pool(name="mpool", bufs=MBUFS) as mp,
            tc.tile_pool(name="spool", bufs=4) as sp,
            tc.tile_pool(name="ssb", bufs=2) as ssb,
            tc.tile_pool(name="hsb", bufs=2) as hsb,
            tc.tile_pool(name="gsb", bufs=2) as gsbp,
            tc.tile_pool(name="psps", bufs=3, space="PSUM") as psps,
            tc.tile_pool(name="psh", bufs=2, space="PSUM") as psh,
            tc.tile_pool(name="psg", bufs=2, space="PSUM") as psg,
            tc.tile_pool(name="psi", bufs=1, space="PSUM") as psi_pool,
        ):
            idx_raw = res.tile([16, NCALLS * IDXCOLS], f32)
            nc.sync.dma_start(idx_raw[:], idx[:, :])
            pat_t = res.tile([16, 128], f32)
            nc.sync.dma_start(pat_t[:], pat[:, :])
            # replicate the 16-partition wrapped index stream to 128 partitions
            # on the PE (out[p,c] = idx_raw[p%16,c]) instead of shipping the
            # 8x-replicated tile through the DMA engines.
            NIREP = (NCALLS * IDXCOLS) // 512
            idx_ts = [res.tile([128, 512], mybir.dt.int16, name=f"idxr{j}")
                      for j in range(NIREP)]

            def rep_chunk(j):
                psi = psi_pool.tile([128, 512], f32, tag="psi")
                nc.tensor.matmul(out=psi[:], lhsT=pat_t[:],
                                 rhs=idx_raw[:, j * 512:(j + 1) * 512],
                                 start=True, stop=True)
                nc.scalar.activation(idx_ts[j][:], psi[:],
                                     mybir.ActivationFunctionType.Copy)

            for j in range(3):
                rep_chunk(j)
            meta_t = res.tile([128, 2 * NBLOCKS], f32)
            nc.sync.dma_start(meta_t[:], meta[:, :])
            iota_t = res.tile([128, DTILE], bf16)
            nc.sync.dma_start(iota_t[:], iota[:, :])
            self_t = res.tile([128, NSLOTS], bf16)
            nc.sync.dma_start(self_t[:], selfp[:, :])
            w1t = res.tile([128, D_HID], bf16)
            nc.sync.dma_start(w1t[:], w1[:, :])
            w2t = res.tile([128, D_HID], bf16)
            nc.sync.dma_start(w2t[:, 0:128], w2[0:128, :])
            nc.sync.dma_start(w2t[:, 128:256], w2[128:256, :])
            b1t = res.tile([128, 2], f32)
            nc.sync.dma_start(b1t[:], b1c[:, :])

            iota_ap = iota_t[:]

            for q in range(GROUPS):
                jnext = q // 2 + 3
                if q % 2 == 0 and jnext < NIREP:
                    rep_chunk(jnext)
                ms = []
                for h in range(2):
                    call = q * 2 + h
                    m = mp.tile([128, CALL_IDX], bf16, tag="m")
                    nc.gpsimd.dma_gather(
                        out_ap=m[:].rearrange("p (b e) -> p b e", e=D_IN),
                        in_ap=tbl[h * HALF:(h + 1) * HALF, :],
                        idxs_ap=idx_ts[call // 4][:, (call % 4) * IDXCOLS:(call % 4 + 1) * IDXCOLS],
                        num_idxs=CALL_IDX,
                        num_idxs_reg=CALL_IDX,
                        elem_size=D_IN,
                        single_packet=False,
                        queue_num=call % 4,
                    )
                    ms.append(m)
                s_sb = ssb.tile([128, 256], bf16, tag="s_sb")
                for bb in range(2):
                    r = q * 2 + bb
                    ps = psps.tile([128, DTILE], f32, tag="ps")
                    for h in range(2):
                        for b in range(NBLK_H):
                            blkcol = ((q * 2 + h) * 2 + bb) * NBLK_H + b
                            s = sp.tile([128, DTILE], bf16, tag="s")
                            nc.vector.tensor_scalar(
                                out=s[:],
                                in0=iota_ap,
                                scalar1=meta_t[:, 2 * blkcol:2 * blkcol + 1],
                                scalar2=meta_t[:, 2 * blkcol + 1:2 * blkcol + 2],
                                op0=mybir.AluOpType.is_equal,
                                op1=mybir.AluOpType.mult,
                            )
                            nc.tensor.matmul(
                                out=ps[:],
                                lhsT=ms[h][:, (bb * NBLK_H + b) * 128:(bb * NBLK_H + b + 1) * 128],
                                rhs=s[:],
                                start=(h == 0 and b == 0),
                                stop=(h == 1 and b == NBLK_H - 1),
                            )
                    nc.vector.tensor_tensor(
                        out=s_sb[:, bb * DTILE:(bb + 1) * DTILE],
                        in0=ps[:],
                        in1=self_t[:, r * DTILE:(r + 1) * DTILE],
                        op=mybir.AluOpType.add,
                    )

                h1ps = psh.tile([128, 512], f32, tag="h1ps")
                nc.tensor.matmul(out=h1ps[:, 0:256], lhsT=w1t[:, 0:128], rhs=s_sb[:], start=True, stop=True)
                nc.tensor.matmul(out=h1ps[:, 256:512], lhsT=w1t[:, 128:256], rhs=s_sb[:], start=True, stop=True)
                h1 = hsb.tile([128, 512], bf16, tag="h1")
                nc.scalar.activation(h1[:, 0:256], h1ps[:, 0:256], mybir.ActivationFunctionType.Relu, bias=b1t[:, 0:1])
                nc.scalar.activation(h1[:, 256:512], h1ps[:, 256:512], mybir.ActivationFunctionType.Relu, bias=b1t[:, 1:2])
                gps = psg.tile([128, 256], f32, tag="gps")
                nc.tensor.matmul(out=gps[:], lhsT=w2t[:, 0:128], rhs=h1[:, 0:256], start=True, stop=False)
                nc.tensor.matmul(out=gps[:], lhsT=w2t[:, 128:256], rhs=h1[:, 256:512], start=False, stop=True)
                gsb = gsbp.tile([128, 256], bf16, tag="gsb")
                nc.vector.tensor_copy(gsb[:], gps[:])
                nc.sync.dma_start(gout[q], gsb[:])

    nc.compile()
    return nc


def _get_program():
    global _prog
    if _prog is None:
        _prog = _build_program()
    return _prog


def _pack_core(deg2):
    """Greedy balance 12,500 nodes into R bins: <=DTILE nodes/bin and both
    per-half edge loads <= CAP_H. deg2: [NPC, 2]."""
    tot = deg2[:, 0] + deg2[:, 1]
    order = np.argsort(-tot, kind="stable")
    binload = np.zeros((R, 2), np.int64)
    bincnt = np.zeros(R, np.int64)
    bin_of = np.full(NPC, -1, np.int32)
    for v in order:
        d0, d1 = deg2[v]
        ok = (bincnt < DTILE) & (binload[:, 0] + d0 <= CAP_H) & (binload[:, 1] + d1 <= CAP_H)
        if not ok.any():
            return None
        score = np.maximum(binload[:, 0] + d0, binload[:, 1] + d1)
        score = np.where(ok, score, 1 << 30)
        b = int(np.argmin(score))
        bin_of[v] = b
        bincnt[b] += 1
        binload[b, 0] += d0
        binload[b, 1] += d1
    return bin_of


def _preprocess_graph(edge):
    """Per graph: per-core packing + slot assembly. Returns (cores, dinv)."""
    src = np.asarray(edge[0], np.int64)
    dst = np.asarray(edge[1], np.int64)
    deg = np.bincount(dst, minlength=N_NODES).astype(np.float32)
    dinv = (1.0 / np.sqrt(deg + np.float32(1.0))).astype(np.float32)
    anorm = (dinv[src] * dinv[dst]).astype(np.float32)
    ehalf = (src >= HALF).astype(np.int64)

    cores = []
    for c in range(CORES_PER_GRAPH):
        lo, hi = c * NPC, (c + 1) * NPC
        emask = (dst >= lo) & (dst < hi)
        es = src[emask]
        ed = dst[emask] - lo
        en = anorm[emask]
        eh = ehalf[emask]

        deg2 = np.zeros((NPC, 2), np.int64)
        np.add.at(deg2, (ed, eh), 1)
        bin_of = _pack_core(deg2)
        assert bin_of is not None, "bin packing failed"

        # position of each node within its bin
        order2 = np.lexsort((np.arange(NPC), bin_of))
        pos_in_bin = np.empty(NPC, np.int64)
        binstart = np.zeros(R + 1, np.int64)
        np.cumsum(np.bincount(bin_of, minlength=R), out=binstart[1:])
        pos_in_bin[order2] = np.arange(NPC) - binstart[bin_of[order2]]

        # column map: group q, col -> global node id (or -1); self slot rows
        cols_map = np.full((GROUPS, 256), -1, np.int64)
        q_of = bin_of // 2
        col_of = (bin_of % 2) * DTILE + pos_in_bin
        cols_map[q_of, col_of] = np.arange(lo, hi)
        self_rows = np.full(NSLOTS, -1, np.int64)
        self_rows[bin_of * DTILE + pos_in_bin] = np.arange(lo, hi)

        # --- edge slot assembly: stream k = ((q*2 + h)*2 + bb), cap 1024 ---
        k = (q_of[ed] * 2 + eh) * 2 + (bin_of[ed] % 2)
        okey = np.lexsort((np.arange(len(es)), k))
        ksorted = k[okey]
        counts = np.bincount(ksorted, minlength=R * 2)
        assert counts.max() <= CAP_H
        starts = np.zeros(R * 2 + 1, np.int64)
        np.cumsum(counts, out=starts[1:])
        within = np.arange(len(es)) - starts[ksorted]
        slot = ksorted * CAP_H + within

        # pad slots gather row 0 with dstrel=-1 (S column contribution 0)
        total = NCALLS * CALL_IDX
        idx_slots = np.zeros(total, np.int64)
        dst_slots = np.full(total, -1.0, np.float32)
        nrm_slots = np.zeros(total, np.float32)
        idx_slots[slot] = es[okey] - eh[okey] * HALF
        dst_slots[slot] = pos_in_bin[ed[okey]].astype(np.float32)
        nrm_slots[slot] = en[okey]

        a = idx_slots.reshape(NCALLS, IDXCOLS, 16)
        idx16 = np.ascontiguousarray(
            np.transpose(a, (2, 0, 1)).reshape(16, NCALLS * IDXCOLS)).astype(np.float32)
        pair = np.stack([dst_slots.reshape(NBLOCKS, 128),
                         nrm_slots.reshape(NBLOCKS, 128)], axis=1)  # [NBLOCKS, 2, 128]
        meta = np.ascontiguousarray(pair.reshape(NBLOCKS * 2, 128).T).astype(np.float32)

        cores.append({
            "idx": idx16, "meta": meta, "cols_map": cols_map,
            "self_rows": self_rows, "lo": lo,
        })
    return cores, dinv


def _self_block(core, tbl_f32, dinv):
    """[128, NSLOTS] bf16: column (bin*128+pos) = dinv^2[v] * tbl[v]."""
    sr = core["self_rows"]
    valid = sr >= 0
    blk = np.zeros((NSLOTS, D_IN), np.float32)
    v = sr[valid]
    blk[valid] = tbl_f32[v] * (dinv[v] * dinv[v])[:, None]
    return np.ascontiguousarray(blk.T).astype(BF16)


def _assemble(results, cores_list):
    """Gather per-core gout into full [N, 128] fp32 arrays for each graph."""
    outs = []
    for g, cores in enumerate(cores_list):
        full = np.zeros((N_NODES, D_IN), np.float32)
        for c in range(CORES_PER_GRAPH):
            go = np.asarray(results[g * CORES_PER_GRAPH + c]["gout"], dtype=np.float32)
            cm = cores[c]["cols_map"]
            for q in range(GROUPS):
                valid = cm[q] >= 0
                full[cm[q][valid]] = go[q][:, valid].T
        outs.append(full)
    return outs


def _spot_check(full, tbl, edge, dinv, post, n_samples=24, tol=5e-2):
    """Verify a few random nodes of a launch output on host (numpy)."""
    src = np.asarray(edge[0], np.int64)
    dst = np.asarray(edge[1], np.int64)
    rng = np.random.default_rng(12345)
    nodes = rng.integers(0, N_NODES, size=n_samples)
    for v in nodes:
        ine = np.where(dst == v)[0]
        s = (dinv[src[ine]] * dinv[v])[:, None] * tbl[src[ine]]
        s = s.sum(axis=0, dtype=np.float64) + np.float64(dinv[v]) ** 2 * tbl[v]
        exp = post(s)
        got = full[v]
        scale = max(np.abs(exp).max(), 1e-3)
        if np.abs(got - exp).max() / scale > tol:
            return False
    return True


LAUNCH_WALL = []
IOTA = np.ascontiguousarray(
    np.broadcast_to(np.arange(DTILE, dtype=np.float32), (128, DTILE))).astype(BF16)
PAT = (np.arange(128)[None, :] % 16 == np.arange(16)[:, None]).astype(np.float32)


def kernel(x_a, edge_a, x_b, edge_b, W1, b1, W2, b2):
    x_a = np.ascontiguousarray(np.asarray(x_a, np.float32))
    x_b = np.ascontiguousarray(np.asarray(x_b, np.float32))
    W1 = np.asarray(W1, np.float32)
    b1 = np.asarray(b1, np.float32)
    W2 = np.asarray(W2, np.float32)
    b2 = np.asarray(b2, np.float32)

    nc = _get_program()
    cores_a, dinv_a = _preprocess_graph(np.asarray(edge_a))
    cores_b, dinv_b = _preprocess_graph(np.asarray(edge_b))

    b1c = np.stack([b1[0:128], b1[128:256]], axis=1).astype(np.float32)
    eye = np.eye(128, dtype=np.float32)
    w1_id = np.concatenate([eye, np.zeros((128, 128), np.float32)], axis=1).astype(BF16)
    w2_id = np.concatenate([eye, np.zeros((128, 128), np.float32)], axis=0).astype(BF16)
    b1c_id = np.stack([b2, np.zeros(128, np.float32)], axis=1).astype(np.float32)
    w1_b = W1.astype(BF16)
    w2_b = W2.astype(BF16)

    def maps(tbl_a, tbl_b, w1m, w2m, b1m):
        tba = tbl_a.astype(BF16)
        tbb = tbl_b.astype(BF16)
        ms = []
        for tb, tf, cores, dinv in ((tba, tbl_a, cores_a, dinv_a),
                                    (tbb, tbl_b, cores_b, dinv_b)):
            for c in range(CORES_PER_GRAPH):
                ms.append({
                    "tbl": tb,
                    "idx": cores[c]["idx"],
                    "meta": cores[c]["meta"],
                    "iota": IOTA,
                    "pat": PAT,
                    "selfp": _self_block(cores[c], tf, dinv),
                    "w1": w1m, "w2": w2m, "b1c": b1m,
                })
        return ms

    core_ids = list(range(N_CORES))

    def run(in_maps):
        import time as _t
        last = None
        for attempt in range(4):
            try:
                t0 = _t.time()
                res = run_bass_kernel_spmd(nc, in_maps, core_ids)
                LAUNCH_WALL.append(_t.time() - t0)
                return res
            except Exception as e:  # wedged core recovers on retry
                last = e
                _t.sleep(5)
        raise last

    def post_a(s):
        return np.maximum(s @ W1.astype(np.float64) + b1, 0.0) @ W2.astype(np.float64)

    def post_b(s):
        return np.maximum(s + b2, 0.0)

    # run each launch until the host spot-check passes (guards against rare
    # silent device-side corruption)
    for attempt in range(4):
        resA = run(maps(x_a, x_b, w1_b, w2_b, b1c))
        g_a, g_b = _assemble(resA.results, (cores_a, cores_b))
        if (_spot_check(g_a, x_a, edge_a, dinv_a, post_a)
                and _spot_check(g_b, x_b, edge_b, dinv_b, post_a)):
            break
    for attempt in range(4):
        resB = run(maps(g_a, g_b, w1_id, w2_id, b1c_id))
        z_a, z_b = _assemble(resB.results, (cores_a, cores_b))
        if (_spot_check(z_a, g_a, edge_a, dinv_a, post_b)
                and _spot_check(z_b, g_b, edge_b, dinv_b, post_b)):
            break
    return (z_a, z_b)


# revision 4
# speedup vs baseline: 1.9697x; 1.0033x over previous
"""2-layer GCN encoder on two graphs, distributed over 8 Trainium2 NeuronCores.

Strategy (v2)
-------------
Graph a -> cores 0-3, graph b -> cores 4-7. Each core owns 12,500 destination
nodes (original-id contiguous chunk) packed into 100 bins of <=128 dst slots,
balanced so every (bin, src-half) in-edge load fits 1024 slots (8 blocks of
128). Self-loops are NOT materialized as edges: the self term is added from a
host-prescaled transposed own-chunk block (dinv^2 * x_own)^T resident in SBUF.

Gathers are batched: ONE dma_gather per (group of 2 bins, half) = 2048 rows of
256 B (bf16), so SWDGE descriptor-gen fixed cost (994 ns/call) amortizes over
2048 descriptors. Tables are bf16, aggregation matmuls run bf16 (1 cyc/col vs
4 for fp32), psum accumulates fp32.

Per 128-edge block the core builds a [128 e x 128 dst] norm-scaled selection
matrix on the vector engine (is_equal*norm against an iota row) and
accumulates M^T @ S -> psum[feat, dst]. Two bins form a 256-column group that
flows through the dense chain W1 -> (+b1) relu -> W2 on chip (all bf16 in,
fp32 psum). The same compiled program serves both GCN layers:
  launch A: table = x  (bf16), weights (W1, b1, W2)       -> g
  launch B: table = g  (bf16), weights ([I|0], b2, [I;0]) -> z = relu(agg+b2)
using A_hat (x W) == (A_hat x) W so the sparse part always runs at 128
features. Host does packing / index prep / unpermute only.
"""

import os
import numpy as np

os.environ.setdefault("JAX_COMPILATION_CACHE_DIR", "/tmp/jax_cache")

import jax  # noqa: E402

try:
    jax.config.update("jax_compilation_cache_dir", "/tmp/jax_cache")
    jax.config.update("jax_persistent_cache_min_compile_time_secs", 0.0)
except Exception:
    pass

import ml_dtypes  # noqa: E402
import concourse.bacc as bacc  # noqa: E402
import concourse.tile as tile  # noqa: E402
import concourse.mybir as mybir  # noqa: E402
from concourse.bass_utils import run_bass_kernel_spmd  # noqa: E402

# ---- static problem geometry (hardcoded per contract) ----
N_NODES = 50000
D_IN = 128
D_HID = 256
HALF = 25000

N_CORES = 8
CORES_PER_GRAPH = 4
NPC = N_NODES // CORES_PER_GRAPH  # 12500 nodes per core

R = 100                    # bins per core
DTILE = 128                # dst slots per bin
NBLK_H = 8                 # 128-edge blocks per (bin, half)
CAP_H = NBLK_H * 128       # 1024 edge slots per (bin, half)
GROUPS = R // 2            # 50 dense groups of 2 bins (256 dst slots)
NCALLS = GROUPS * 2        # one dma_gather per (group, half): 2048 idx
CALL_IDX = 2 * CAP_H       # 2048 rows per gather call
IDXCOLS = CALL_IDX // 16   # 128 int16 cols per call
NBLOCKS = R * 2 * NBLK_H   # 1600 S-blocks per core
NSLOTS = R * DTILE         # 12800 dst slots per core
MBUFS = 6

BF16 = ml_dtypes.bfloat16

_prog = None


def _build_program():
    nc = bacc.Bacc("TRN2", target_bir_lowering=False, num_swdge_queues=4)
    f32 = mybir.dt.float32
    bf16 = mybir.dt.bfloat16
    tbl = nc.declare_dram_parameter("tbl", [N_NODES, D_IN], bf16, isOutput=False)
    idx = nc.declare_dram_parameter("idx", [16, NCALLS * IDXCOLS], f32, isOutput=False)
    pat = nc.declare_dram_parameter("pat", [16, 128], f32, isOutput=False)
    meta = nc.declare_dram_parameter("meta", [128, 2 * NBLOCKS], f32, isOutput=False)  # interleaved (dstrel, norm) per block
    iota = nc.declare_dram_parameter("iota", [128, DTILE], bf16, isOutput=False)
    selfp = nc.declare_dram_parameter("selfp", [128, NSLOTS], bf16, isOutput=False)
    w1 = nc.declare_dram_parameter("w1", [128, D_HID], bf16, isOutput=False)
    w2 = nc.declare_dram_parameter("w2", [D_HID, 128], bf16, isOutput=False)
    b1c = nc.declare_dram_parameter("b1c", [128, 2], f32, isOutput=False)
    gout = nc.declare_dram_parameter("gout", [GROUPS, 128, 256], bf16, isOutput=True)

    with tile.TileContext(nc) as tc:
        with (
            tc.tile_pool(name="res", bufs=1) as res,
            tc.tile_pool(name="mpool", bufs=MBUFS) as mp,
            tc.tile_pool(name="spool", bufs=4) as sp,
            tc.tile_pool(name="ssb", bufs=2) as ssb,
            tc.tile_pool(name="hsb", bufs=2) as hsb,
            tc.tile_pool(name="gsb", bufs=2) as gsbp,
            tc.tile_pool(name="psps", bufs=3, space="PSUM") as psps,
            tc.tile_pool(name="psh", bufs=2, space="PSUM") as psh,
            tc.tile_pool(name="psg", bufs=2, space="PSUM") as psg,
            tc.tile_pool(name="psi", bufs=1, space="PSUM") as psi_pool,
        ):
            idx_raw = res.tile([16, NCALLS * IDXCOLS], f32)
            nc.sync.dma_start(idx_raw[:], idx[:, :])
            pat_t = res.tile([16, 128], f32)
            nc.sync.dma_start(pat_t[:], pat[:, :])
            # replicate the 16-partition wrapped index stream to 128 partitions
            # on the PE (out[p,c] = idx_raw[p%16,c]) instead of shipping the
            # 8x-replicated tile through the DMA engines.
            NIREP = (NCALLS * IDXCOLS) // 512
            idx_ts = [res.tile([128, 512], mybir.dt.int16, name=f"idxr{j}")
                      for j in range(NIREP)]

            def rep_chunk(j):
                psi = psi_pool.tile([128, 512], f32, tag="psi")
                nc.tensor.matmul(out=psi[:], lhsT=pat_t[:],
                                 rhs=idx_raw[:, j * 512:(j + 1) * 512],
                                 start=True, stop=True)
                nc.scalar.activation(idx_ts[j][:], psi[:],
                                     mybir.ActivationFunctionType.Copy)

            for j in range(3):
                rep_chunk(j)
            meta_t = res.tile([128, 2 * NBLOCKS], f32)
            nc.sync.dma_start(meta_t[:], meta[:, :])
            iota_t = res.tile([128, DTILE], bf16)
            nc.sync.dma_start(iota_t[:], iota[:, :])
            self_t = res.tile([128, NSLOTS], bf16)
            nc.sync.dma_start(self_t[:], selfp[:, :])
            w1t = res.tile([128, D_HID], bf16)
            nc.sync.dma_start(w1t[:], w1[:, :])
            w2t = res.tile([128, D_HID], bf16)
            nc.sync.dma_start(w2t[:, 0:128], w2[0:128, :])
            nc.sync.dma_start(w2t[:, 128:256], w2[128:256, :])
            b1t = res.tile([128, 2], f32)
            nc.sync.dma_start(b1t[:], b1c[:, :])

            iota_ap = iota_t[:]

            for q in range(GROUPS):
                jnext = q // 2 + 3
                if q % 2 == 0 and jnext < NIREP:
                    rep_chunk(jnext)
                ms = []
                for h in range(2):
                    call = q * 2 + h
                    m = mp.tile([128, CALL_IDX], bf16, tag="m")
                    nc.gpsimd.dma_gather(
                        out_ap=m[:].rearrange("p (b e) -> p b e", e=D_IN),
                        in_ap=tbl[h * HALF:(h + 1) * HALF, :],
                        idxs_ap=idx_ts[call // 4][:, (call % 4) * IDXCOLS:(call % 4 + 1) * IDXCOLS],
                        num_idxs=CALL_IDX,
                        num_idxs_reg=CALL_IDX,
                        elem_size=D_IN,
                        single_packet=False,
                        queue_num=call % 4,
                    )
                    ms.append(m)
                s_sb = ssb.tile([128, 256], bf16, tag="s_sb")
                for bb in range(2):
                    r = q * 2 + bb
                    ps = psps.tile([128, DTILE], f32, tag="ps")
                    for h in range(2):
                        for b in range(NBLK_H):
                            blkcol = ((q * 2 + h) * 2 + bb) * NBLK_H + b
                            s = sp.tile([128, DTILE], bf16, tag="s")
                            nc.vector.tensor_scalar(
                                out=s[:],
                                in0=iota_ap,
                                scalar1=meta_t[:, 2 * blkcol:2 * blkcol + 1],
                                scalar2=meta_t[:, 2 * blkcol + 1:2 * blkcol + 2],
                                op0=mybir.AluOpType.is_equal,
                                op1=mybir.AluOpType.mult,
                            )
                            nc.tensor.matmul(
                                out=ps[:],
                                lhsT=ms[h][:, (bb * NBLK_H + b) * 128:(bb * NBLK_H + b + 1) * 128],
                                rhs=s[:],
                                start=(h == 0 and b == 0),
                                stop=(h == 1 and b == NBLK_H - 1),
                            )
                    nc.vector.tensor_tensor(
                        out=s_sb[:, bb * DTILE:(bb + 1) * DTILE],
                        in0=ps[:],
                        in1=self_t[:, r * DTILE:(r + 1) * DTILE],
                        op=mybir.AluOpType.add,
                    )

                h1ps = psh.tile([128, 512], f32, tag="h1ps")
                nc.tensor.matmul(out=h1ps[:, 0:256], lhsT=w1t[:, 0:128], rhs=s_sb[:], start=True, stop=True)
                nc.tensor.matmul(out=h1ps[:, 256:512], lhsT=w1t[:, 128:256], rhs=s_sb[:], start=True, stop=True)
                h1 = hsb.tile([128, 512], bf16, tag="h1")
                nc.scalar.activation(h1[:, 0:256], h1ps[:, 0:256], mybir.ActivationFunctionType.Relu, bias=b1t[:, 0:1])
                nc.scalar.activation(h1[:, 256:512], h1ps[:, 256:512], mybir.ActivationFunctionType.Relu, bias=b1t[:, 1:2])
                gps = psg.tile([128, 256], f32, tag="gps")
                nc.tensor.matmul(out=gps[:], lhsT=w2t[:, 0:128], rhs=h1[:, 0:256], start=True, stop=False)
                nc.tensor.matmul(out=gps[:], lhsT=w2t[:, 128:256], rhs=h1[:, 256:512], start=False, stop=True)
                gsb = gsbp.tile([128, 256], bf16, tag="gsb")
                nc.scalar.activation(gsb[:], gps[:], mybir.ActivationFunctionType.Copy)
                nc.sync.dma_start(gout[q], gsb[:])

    nc.compile()
    return nc


def _get_program():
    global _prog
    if _prog is None:
        _prog = _build_program()
    return _prog


def _pack_core(deg2):
    """Greedy balance 12,500 nodes into R bins: <=DTILE nodes/bin and both
    per-half edge loads <= CAP_H. deg2: [NPC, 2]."""
    tot = deg2[:, 0] + deg2[:, 1]
    order = np.argsort(-tot, kind="stable")
    binload = np.zeros((R, 2), np.int64)
    bincnt = np.zeros(R, np.int64)
    bin_of = np.full(NPC, -1, np.int32)
    for v in order:
        d0, d1 = deg2[v]
        ok = (bincnt < DTILE) & (binload[:, 0] + d0 <= CAP_H) & (binload[:, 1] + d1 <= CAP_H)
        if not ok.any():
            return None
        score = np.maximum(binload[:, 0] + d0, binload[:, 1] + d1)
        score = np.where(ok, score, 1 << 30)
        b = int(np.argmin(score))
        bin_of[v] = b
        bincnt[b] += 1
        binload[b, 0] += d0
        binload[b, 1] += d1
    return bin_of


def _preprocess_graph(edge):
    """Per graph: per-core packing + slot assembly. Returns (cores, dinv)."""
    src = np.asarray(edge[0], np.int64)
    dst = np.asarray(edge[1], np.int64)
    deg = np.bincount(dst, minlength=N_NODES).astype(np.float32)
    dinv = (1.0 / np.sqrt(deg + np.float32(1.0))).astype(np.float32)
    anorm = (dinv[src] * dinv[dst]).astype(np.float32)
    ehalf = (src >= HALF).astype(np.int64)

    cores = []
    for c in range(CORES_PER_GRAPH):
        lo, hi = c * NPC, (c + 1) * NPC
        emask = (dst >= lo) & (dst < hi)
        es = src[emask]
        ed = dst[emask] - lo
        en = anorm[emask]
        eh = ehalf[emask]

        deg2 = np.zeros((NPC, 2), np.int64)
        np.add.at(deg2, (ed, eh), 1)
        bin_of = _pack_core(deg2)
        assert bin_of is not None, "bin packing failed"

        # position of each node within its bin
        order2 = np.lexsort((np.arange(NPC), bin_of))
        pos_in_bin = np.empty(NPC, np.int64)
        binstart = np.zeros(R + 1, np.int64)
        np.cumsum(np.bincount(bin_of, minlength=R), out=binstart[1:])
        pos_in_bin[order2] = np.arange(NPC) - binstart[bin_of[order2]]

        # column map: group q, col -> global node id (or -1); self slot rows
        cols_map = np.full((GROUPS, 256), -1, np.int64)
        q_of = bin_of // 2
        col_of = (bin_of % 2) * DTILE + pos_in_bin
        cols_map[q_of, col_of] = np.arange(lo, hi)
        self_rows = np.full(NSLOTS, -1, np.int64)
        self_rows[bin_of * DTILE + pos_in_bin] = np.arange(lo, hi)

        # --- edge slot assembly: stream k = ((q*2 + h)*2 + bb), cap 1024 ---
        k = (q_of[ed] * 2 + eh) * 2 + (bin_of[ed] % 2)
        okey = np.lexsort((np.arange(len(es)), k))
        ksorted = k[okey]
        counts = np.bincount(ksorted, minlength=R * 2)
        assert counts.max() <= CAP_H
        starts = np.zeros(R * 2 + 1, np.int64)
        np.cumsum(counts, out=starts[1:])
        within = np.arange(len(es)) - starts[ksorted]
        slot = ksorted * CAP_H + within

        # pad slots gather row 0 with dstrel=-1 (S column contribution 0)
        total = NCALLS * CALL_IDX
        idx_slots = np.zeros(total, np.int64)
        dst_slots = np.full(total, -1.0, np.float32)
        nrm_slots = np.zeros(total, np.float32)
        idx_slots[slot] = es[okey] - eh[okey] * HALF
        dst_slots[slot] = pos_in_bin[ed[okey]].astype(np.float32)
        nrm_slots[slot] = en[okey]

        a = idx_slots.reshape(NCALLS, IDXCOLS, 16)
        idx16 = np.ascontiguousarray(
            np.transpose(a, (2, 0, 1)).reshape(16, NCALLS * IDXCOLS)).astype(np.float32)
        pair = np.stack([dst_slots.reshape(NBLOCKS, 128),
                         nrm_slots.reshape(NBLOCKS, 128)], axis=1)  # [NBLOCKS, 2, 128]
        meta = np.ascontiguousarray(pair.reshape(NBLOCKS * 2, 128).T).astype(np.float32)

        cores.append({
            "idx": idx16, "meta": meta, "cols_map": cols_map,
            "self_rows": self_rows, "lo": lo,
        })
    return cores, dinv


def _self_block(core, tbl_f32, dinv):
    """[128, NSLOTS] bf16: column (bin*128+pos) = dinv^2[v] * tbl[v]."""
    sr = core["self_rows"]
    valid = sr >= 0
    blk = np.zeros((NSLOTS, D_IN), np.float32)
    v = sr[valid]
    blk[valid] = tbl_f32[v] * (dinv[v] * dinv[v])[:, None]
    return np.ascontiguousarray(blk.T).astype(BF16)


def _assemble(results, cores_list):
    """Gather per-core gout into full [N, 128] fp32 arrays for each graph."""
    outs = []
    for g, cores in enumerate(cores_list):
        full = np.zeros((N_NODES, D_IN), np.float32)
        for c in range(CORES_PER_GRAPH):
            go = np.asarray(results[g * CORES_PER_GRAPH + c]["gout"], dtype=np.float32)
            cm = cores[c]["cols_map"]
            for q in range(GROUPS):
                valid = cm[q] >= 0
                full[cm[q][valid]] = go[q][:, valid].T
        outs.append(full)
    return outs


def _spot_check(full, tbl, edge, dinv, post, n_samples=24, tol=5e-2):
    """Verify a few random nodes of a launch output on host (numpy)."""
    src = np.asarray(edge[0], np.int64)
    dst = np.asarray(edge[1], np.int64)
    rng = np.random.default_rng(12345)
    nodes = rng.integers(0, N_NODES, size=n_samples)
    for v in nodes:
        ine = np.where(dst == v)[0]
        s = (dinv[src[ine]] * dinv[v])[:, None] * tbl[src[ine]]
        s = s.sum(axis=0, dtype=np.float64) + np.float64(dinv[v]) ** 2 * tbl[v]
        exp = post(s)
        got = full[v]
        scale = max(np.abs(exp).max(), 1e-3)
        if np.abs(got - exp).max() / scale > tol:
            return False
    return True


LAUNCH_WALL = []
IOTA = np.ascontiguousarray(
    np.broadcast_to(np.arange(DTILE, dtype=np.float32), (128, DTILE))).astype(BF16)
PAT = (np.arange(128)[None, :] % 16 == np.arange(16)[:, None]).astype(np.float32)


def kernel(x_a, edge_a, x_b, edge_b, W1, b1, W2, b2):
    x_a = np.ascontiguousarray(np.asarray(x_a, np.float32))
    x_b = np.ascontiguousarray(np.asarray(x_b, np.float32))
    W1 = np.asarray(W1, np.float32)
    b1 = np.asarray(b1, np.float32)
    W2 = np.asarray(W2, np.float32)
    b2 = np.asarray(b2, np.float32)

    nc = _get_program()
    cores_a, dinv_a = _preprocess_graph(np.asarray(edge_a))
    cores_b, dinv_b = _preprocess_graph(np.asarray(edge_b))

    b1c = np.stack([b1[0:128], b1[128:256]], axis=1).astype(np.float32)
    eye = np.eye(128, dtype=np.float32)
    w1_id = np.concatenate([eye, np.zeros((128, 128), np.float32)], axis=1).astype(BF16)
    w2_id = np.concatenate([eye, np.zeros((128, 128), np.float32)], axis=0).astype(BF16)
    b1c_id = np.stack([b2, np.zeros(128, np.float32)], axis=1).astype(np.float32)
    w1_b = W1.astype(BF16)
    w2_b = W2.astype(BF16)

    def maps(tbl_a, tbl_b, w1m, w2m, b1m):
        tba = tbl_a.astype(BF16)
        tbb = tbl_b.astype(BF16)
        ms = []
        for tb, tf, cores, dinv in ((tba, tbl_a, cores_a, dinv_a),
                                    (tbb, tbl_b, cores_b, dinv_b)):
            for c in range(CORES_PER_GRAPH):
                ms.append({
                    "tbl": tb,
                    "idx": cores[c]["idx"],
                    "meta": cores[c]["meta"],
                    "iota": IOTA,
                    "pat": PAT,
                    "selfp": _self_block(cores[c], tf, dinv),
                    "w1": w1m, "w2": w2m, "b1c": b1m,
                })
        return ms

    core_ids = list(range(N_CORES))

    def run(in_maps):
        import time as _t
        last = None
        for attempt in range(4):
            try:
                t0 = _t.time()
                res = run_bass_kernel_spmd(nc, in_maps, core_ids)
                LAUNCH_WALL.append(_t.time() - t0)
                return res
            except Exception as e:  # wedged core recovers on retry
                last = e
                _t.sleep(5)
        raise last

    def post_a(s):
        return np.maximum(s @ W1.astype(np.float64) + b1, 0.0) @ W2.astype(np.float64)

    def post_b(s):
        return np.maximum(s + b2, 0.0)

    # run each launch until the host spot-check passes (guards against rare
    # silent device-side corruption)
    for attempt in range(4):
        resA = run(maps(x_a, x_b, w1_b, w2_b, b1c))
        g_a, g_b = _assemble(resA.results, (cores_a, cores_b))
        if (_spot_check(g_a, x_a, edge_a, dinv_a, post_a)
                and _spot_check(g_b, x_b, edge_b, dinv_b, post_a)):
            break
    for attempt in range(4):
        resB = run(maps(g_a, g_b, w1_id, w2_id, b1c_id))
        z_a, z_b = _assemble(resB.results, (cores_a, cores_b))
        if (_spot_check(z_a, g_a, edge_a, dinv_a, post_b)
                and _spot_check(z_b, g_b, edge_b, dinv_b, post_b)):
            break
    return (z_a, z_b)
